# revision 1
# baseline (speedup 1.0000x reference)
"""Trainium2 Bass kernel for nn_BlockLinear forward.

Computes y[b, o] = sum_k exp(log_weight[o, k]) * x[b, o*K + k]
for x [16384, 8192] fp32, log_weight [1024, 8] fp32.

Strategy: data-parallel over batch across 8 NeuronCores (2048 rows each).
Per core, 16 tiles of [128, 8192] stream through SBUF.  The fused
multiply + grouped-reduce runs as ONE custom DVE op per tile:

    S[p, t] = cumsum_t(x[p, t] * w[t])        (scan(ADD, Src0*Src1), II=1)

The scan is SEGMENTED in hardware: a hand-grafted SUB_DIM_DONE step
state in the uop FSM drops the CURR feedback for exactly one element at
every page boundary of in0's [P, G, K] access pattern, resetting the
running sum per group of K (verified on HW: zero per-page overhead,
8690ns for 8192 elems, rel err 1.1e-7).  The OUTPUT access pattern has
innermost stride 0 over each group: all K writes land on one address
and the last (the completed group sum) survives — so one instruction
per tile produces the finished y tile, contiguous and compact.

Why custom: the native tensor_tensor_scan is II=2 (its recurrence
chains two ALU stages); a single-stage ADD recurrence over the stage-0
product runs at 1 element/cycle.  Loads ride the Sync HWDGE queue and
stores the ScalarE HWDGE queue so store sem-waits never block load
issues (HWDGE is FIFO per issuing engine).

Per tile: 8.7us DVE vs 10-14.9us DMA (4.5 MiB; rate depends on
neighbor-core HBM phase) -> memory-bound.  Buffering (4 x-tile bufs +
a dedicated tail-quarter pool), a quarter-split w broadcast gating
quarter-scans of the first tile (Tile deps are AP-range-based), and
the w load riding first on the Sync HWDGE FIFO keep the DMA stream
continuous end to end; first scan starts at ~25us, steady cadence
tracks the DMA at ~10.9us/tile, tail quarters at 2.2us.  Measured on
the 8 axon trn2 cores: 201.5-237us across runs depending on HBM
contention phase (final config validated at 212.5us), scale-relative
error 1.1e-7.
"""

import numpy as np

B = 16384
IN_F = 8192
OUT_F = 1024
K = 8
N_CORES = 8
P = 128

_CACHE = {}

_OP_NAME = "SEGSUM_MUL_SCAN_ANT"
_OP2_NAME = "SEGSUM8_RESET_ANT"


def _build_seg_uops(spec, ver):
    """Lower scan(ADD, Src0*Src1) then graft a SUB_DIM_DONE step state that
    drops the CURR feedback for one element — an exact segmented scan that
    resets at every page boundary of in0's [P, S, N] access pattern."""
    import dataclasses

    from concourse import dve_spec as ds
    from concourse.dve_uop import Trigger

    spec_h = ds._hoist_stream_invariant_ops(spec)
    scans = ds._collect(spec_h.body, ds.Scan)
    latches = ds._collect(spec_h.body, ds.Latch)
    placement = ds._build_placement(
        spec_h, scans, ds.N_STAGES[ver], ds.N_LANES[ver]
    )
    states = ds._build_state_machine(spec_h, scans, latches, placement)
    d = placement.node_stage[scans[0]]
    steady_idx = len(states) - 1
    step_idx = steady_idx + 1
    steady = states[steady_idx]
    states[steady_idx] = dataclasses.replace(
        steady,
        trigger=(Trigger.SRC_TENSOR_DONE, Trigger.SUB_DIM_DONE, Trigger.NONE),
        next=(0, step_idx, 0),
    )
    states.append(
        dataclasses.replace(
            steady,
            overrides={
                **steady.overrides,
                d: ds._Stage(ds.AluOp.BYPASS, scans[0].expr),
            },
            trigger=(Trigger.SRC_TENSOR_DONE, Trigger.SUB_DIM_DONE, Trigger.COUNT),
            next=(0, step_idx, steady_idx),
            repeat=1,
        )
    )
    uops = [ds._assemble(st) for st in states]
    for u in uops:
        u.validate(ver)
    return uops


def _register_seg_op():
    """Register the segmented multiply-scan (page-reset) custom DVE op."""
    import dataclasses

    from concourse import dve_ops
    from concourse.dve_spec import AluOp, Spec, Src0, Src1, scan
    from concourse.dve_uop import DveOpSpec

    for op in dve_ops.OPS:
        if op.name == _OP2_NAME:
            return op

    def _ref(in0, in1, s0, s1, imm2):
        p = (
            np.asarray(in0, np.float32)
            * np.asarray(in1, np.float32).reshape(np.asarray(in0).shape)
        ).astype(np.float32)
        return np.cumsum(p, axis=-1, dtype=np.float32)

    spec = Spec(body=scan(AluOp.ADD, Src0 * Src1), reference=_ref)

    @dataclasses.dataclass(frozen=True)
    class _SegDveOp(dve_ops.DveOp):
        def compile(self, ver):
            key = (self.name, ver)
            cached = dve_ops._COMPILE_CACHE.get(key)
            if cached is not None:
                return cached
            result = DveOpSpec(
                name=self.name,
                opcode=dve_ops.get_dve_sub_opcode(self.name),
                uops=_build_seg_uops(self.spec, ver),
                rd1_en=True,
            )
            got = result.sha(ver)
            if self.uops_sha.get(ver) != got:
                raise ValueError(f"{self.name}: uop drift {got}")
            dve_ops._COMPILE_CACHE[key] = result
            return result

    row = dve_ops._CUSTOM_DVE_ROW_BASE + len(dve_ops.OPS)
    shas = {}
    for ver in ("v3", "v4"):
        s = DveOpSpec(
            name=_OP2_NAME, opcode=row, uops=_build_seg_uops(spec, ver), rd1_en=True
        )
        shas[ver] = s.sha(ver)
    op = _SegDveOp(_OP2_NAME, spec, subdim=True, uops_sha=shas)
    dve_ops.OPS.append(op)
    dve_ops.CUSTOM_DVE_SPECS[_OP2_NAME] = spec
    dve_ops._SUB_OPCODE_FOR_NAME[_OP2_NAME] = row
    return op


def _register_custom_op():
    """Register scan(ADD, Src0*Src1) as a custom DVE op (runtime-local)."""
    from concourse import dve_ops
    from concourse.dve_spec import AluOp, Spec, Src0, Src1, _has_src1, lower, scan
    from concourse.dve_uop import DveOpSpec

    for op in dve_ops.OPS:
        if op.name == _OP_NAME:
            return op

    def _ref(in0, in1, s0, s1, imm2):
        p = (np.asarray(in0, np.float32) * np.asarray(in1, np.float32)).astype(
            np.float32
        )
        shp = p.shape
        return (
            np.cumsum(p.reshape(shp[0], -1), axis=1, dtype=np.float32).reshape(shp)
        )

    spec = Spec(body=scan(AluOp.ADD, Src0 * Src1), reference=_ref)
    row = dve_ops._CUSTOM_DVE_ROW_BASE + len(dve_ops.OPS)
    shas = {}
    for ver in ("v3", "v4"):
        s = DveOpSpec(
            name=_OP_NAME, opcode=row, uops=lower(spec, ver=ver), rd1_en=_has_src1(spec)
        )
        shas[ver] = s.sha(ver)
    op = dve_ops.DveOp(_OP_NAME, spec, subdim=False, uops_sha=shas)
    dve_ops.OPS.append(op)
    dve_ops.CUSTOM_DVE_SPECS[_OP_NAME] = spec
    dve_ops._SUB_OPCODE_FOR_NAME[_OP_NAME] = row
    return op


def _build(b_shard, in_f, out_f, n_cores, x_bufs=4, halves=4, n_prologue=0, tail_quarters=4):
    """Build + compile the per-core Bass module (SPMD across n_cores)."""
    from concourse import bacc, tile, mybir

    op = _register_custom_op()
    op2 = _register_seg_op()

    k = K
    n_tiles = b_shard // P
    hw = in_f // halves  # half-tile width (multiple of K)
    hy = hw // k
    f32 = mybir.dt.float32

    nc = bacc.Bacc(
        "TRN2",
        target_bir_lowering=False,
        debug=False,
        enable_asserts=True,
        num_devices=n_cores,
    )
    x_d = nc.dram_tensor("x", [b_shard, in_f], f32, kind="ExternalInput")
    w_d = nc.dram_tensor("w", [1, in_f], f32, kind="ExternalInput")
    y_d = nc.dram_tensor("y", [b_shard, out_f], f32, kind="ExternalOutput")

    with tile.TileContext(nc) as tc:
        with (
            tc.tile_pool(name="consts", bufs=1) as cpool,
            tc.tile_pool(name="work", bufs=x_bufs) as pool,
            tc.tile_pool(name="outs", bufs=3) as ypool,
            tc.tile_pool(name="tailq", bufs=4) as qpool,
        ):
            wb = cpool.tile([P, in_f], f32, tag="w")
            # w first in the Sync HWDGE FIFO: its 32KB completes ~5us
            # earlier than via SWDGE (GpSimd's preamble delays emission),
            # and it only displaces x0's issue by ~0.7us.
            nc.sync.dma_start(out=wb[0:1, :], in_=w_d[:])
            for h in range(halves):
                nc.gpsimd.partition_broadcast(
                    wb[:, h * hw : (h + 1) * hw], wb[0:1, h * hw : (h + 1) * hw]
                )
            def chunk(i, xap, c0, cw):
                """Process columns [c0, c0+cw) of row-block i from AP xap."""
                rows = slice(i * P, (i + 1) * P)
                cg = cw // k  # groups in this chunk
                # One instruction per chunk: segmented multiply-scan with a
                # hardware page reset (SUB_DIM_DONE step state) over in0's
                # [P, cg, K] access pattern.  The out AP has innermost
                # stride 0 over each group's K elements, so the last write
                # (the completed group sum) survives, laid out contiguously.
                yt = ypool.tile([P, cg], f32, tag="s")
                y_view = yt[:].rearrange("p (g o) -> p g o", o=1).broadcast_to(
                    [P, cg, k]
                )
                nc.vector._custom_dve(
                    op2,
                    out=y_view,
                    in0=xap.rearrange("p (g kk) -> p g kk", kk=k),
                    in1=wb[:, c0 : c0 + cw],
                )
                # y stores ride the ScalarE HWDGE queue so their semaphore
                # waits never block the x-load issue stream (HWDGE is FIFO
                # per issuing engine).
                nc.scalar.dma_start(
                    out=y_d[rows, c0 // k : (c0 + cw) // k], in_=yt[:]
                )

            for i in range(n_tiles):
                rows = slice(i * P, (i + 1) * P)
                if i == n_tiles - 1 and tail_quarters > 1:
                    # split the final tile so the post-stream tail is short
                    qw = in_f // tail_quarters
                    for q in range(tail_quarters):
                        xt = qpool.tile([P, qw], f32, tag="xq")
                        nc.sync.dma_start(
                            out=xt[:], in_=x_d[rows, q * qw : (q + 1) * qw]
                        )
                        chunk(i, xt[:], q * qw, qw)
                else:
                    if i < n_prologue:
                        # dedicated startup buffers: extra DMA runway at start
                        xt = cpool.tile([P, in_f], f32, tag=f"xpro{i}")
                    else:
                        xt = pool.tile([P, in_f], f32, tag="x")
                    nc.sync.dma_start(out=xt[:], in_=x_d[rows, :])
                    if i == 0 and halves > 1:
                        # quarter-scans against matching wb ranges: each
                        # gates on its own partial broadcast, starting
                        # compute ~9us earlier (no extra bytes moved)
                        for q in range(halves):
                            chunk(i, xt[:, q * hw : (q + 1) * hw], q * hw, hw)
                    else:
                        chunk(i, xt[:], 0, in_f)
    nc.compile()
    return nc


def _prep_weights(log_weight, out_f, k):
    w = np.exp(np.asarray(log_weight, np.float64)).reshape(1, -1)  # [1, out_f*k]
    return np.ascontiguousarray(w, dtype=np.float32)


def kernel(x, log_weight):
    from concourse import bass_utils

    x = np.ascontiguousarray(np.asarray(x, dtype=np.float32))
    assert x.shape == (B, IN_F), x.shape
    b_shard = B // N_CORES

    if "nc" not in _CACHE:
        _CACHE["nc"] = _build(b_shard, IN_F, OUT_F, N_CORES)
    nc = _CACHE["nc"]

    wb = _prep_weights(log_weight, OUT_F, K)
    in_maps = [
        {"x": x[i * b_shard : (i + 1) * b_shard], "w": wb}
        for i in range(N_CORES)
    ]
    res = bass_utils.run_bass_kernel_spmd(nc, in_maps, core_ids=list(range(N_CORES)))
    y = np.concatenate([res.results[i]["y"] for i in range(N_CORES)], axis=0)
    return y



# revision 7
# speedup vs baseline: 1.1785x; 1.1785x over previous
"""Trainium2 Bass kernel for nn_BlockLinear forward.

Computes y[b, o] = sum_k exp(log_weight[o, k]) * x[b, o*K + k]
for x [16384, 8192] fp32, log_weight [1024, 8] fp32.

Strategy: data-parallel over batch across 8 NeuronCores (2048 rows each).
Per core, 16 tiles of [128, 8192] stream through SBUF.  The fused
multiply + grouped-reduce runs as ONE custom DVE op per tile:

    S[p, t] = cumsum_t(x[p, t] * w[t])        (scan(ADD, Src0*Src1), II=1)

The scan is SEGMENTED in hardware: a hand-grafted SUB_DIM_DONE step
state in the uop FSM drops the CURR feedback for exactly one element at
every page boundary of in0's [P, G, K] access pattern, resetting the
running sum per group of K (verified on HW: zero per-page overhead,
8690ns for 8192 elems, rel err 1.1e-7).  The OUTPUT access pattern has
innermost stride 0 over each group: all K writes land on one address
and the last (the completed group sum) survives — so one instruction
per tile produces the finished y tile, contiguous and compact.

Why custom: the native tensor_tensor_scan is II=2 (its recurrence
chains two ALU stages); a single-stage ADD recurrence over the stage-0
product runs at 1 element/cycle.  Loads ride the Sync HWDGE queue and
stores the ScalarE HWDGE queue so store sem-waits never block load
issues (HWDGE is FIFO per issuing engine).

Per tile: 8.7us DVE vs 10-14.9us DMA (4.5 MiB; rate depends on
neighbor-core HBM phase) -> memory-bound.  Buffering (4 x-tile bufs +
a dedicated tail-quarter pool), a quarter-split w broadcast gating
quarter-scans of the first tile (Tile deps are AP-range-based), and
the w load riding first on the Sync HWDGE FIFO keep the DMA stream
continuous end to end; first scan starts at ~25us, steady cadence
tracks the DMA at ~10.9us/tile, tail quarters at 2.2us.  Measured on
the 8 axon trn2 cores: 201.5-237us across runs depending on HBM
contention phase (final config validated at 212.5us), scale-relative
error 1.1e-7.
"""

import numpy as np

B = 16384
IN_F = 8192
OUT_F = 1024
K = 8
N_CORES = 8
P = 128

_CACHE = {}

_OP_NAME = "SEGSUM_MUL_SCAN_ANT"
_OP2_NAME = "SEGSUM8_RESET_ANT"


def _build_seg_uops(spec, ver):
    """Lower scan(ADD, Src0*Src1) then graft a SUB_DIM_DONE step state that
    drops the CURR feedback for one element — an exact segmented scan that
    resets at every page boundary of in0's [P, S, N] access pattern."""
    import dataclasses

    from concourse import dve_spec as ds
    from concourse.dve_uop import Trigger

    spec_h = ds._hoist_stream_invariant_ops(spec)
    scans = ds._collect(spec_h.body, ds.Scan)
    latches = ds._collect(spec_h.body, ds.Latch)
    placement = ds._build_placement(
        spec_h, scans, ds.N_STAGES[ver], ds.N_LANES[ver]
    )
    states = ds._build_state_machine(spec_h, scans, latches, placement)
    d = placement.node_stage[scans[0]]
    steady_idx = len(states) - 1
    step_idx = steady_idx + 1
    steady = states[steady_idx]
    states[steady_idx] = dataclasses.replace(
        steady,
        trigger=(Trigger.SRC_TENSOR_DONE, Trigger.SUB_DIM_DONE, Trigger.NONE),
        next=(0, step_idx, 0),
    )
    states.append(
        dataclasses.replace(
            steady,
            overrides={
                **steady.overrides,
                d: ds._Stage(ds.AluOp.BYPASS, scans[0].expr),
            },
            trigger=(Trigger.SRC_TENSOR_DONE, Trigger.SUB_DIM_DONE, Trigger.COUNT),
            next=(0, step_idx, steady_idx),
            repeat=1,
        )
    )
    uops = [ds._assemble(st) for st in states]
    for u in uops:
        u.validate(ver)
    return uops


def _register_seg_op():
    """Register the segmented multiply-scan (page-reset) custom DVE op."""
    import dataclasses

    from concourse import dve_ops
    from concourse.dve_spec import AluOp, Spec, Src0, Src1, scan
    from concourse.dve_uop import DveOpSpec

    for op in dve_ops.OPS:
        if op.name == _OP2_NAME:
            return op

    def _ref(in0, in1, s0, s1, imm2):
        p = (
            np.asarray(in0, np.float32)
            * np.asarray(in1, np.float32).reshape(np.asarray(in0).shape)
        ).astype(np.float32)
        return np.cumsum(p, axis=-1, dtype=np.float32)

    spec = Spec(body=scan(AluOp.ADD, Src0 * Src1), reference=_ref)

    @dataclasses.dataclass(frozen=True)
    class _SegDveOp(dve_ops.DveOp):
        def compile(self, ver):
            key = (self.name, ver)
            cached = dve_ops._COMPILE_CACHE.get(key)
            if cached is not None:
                return cached
            result = DveOpSpec(
                name=self.name,
                opcode=dve_ops.get_dve_sub_opcode(self.name),
                uops=_build_seg_uops(self.spec, ver),
                rd1_en=True,
            )
            got = result.sha(ver)
            if self.uops_sha.get(ver) != got:
                raise ValueError(f"{self.name}: uop drift {got}")
            dve_ops._COMPILE_CACHE[key] = result
            return result

    row = dve_ops._CUSTOM_DVE_ROW_BASE + len(dve_ops.OPS)
    shas = {}
    for ver in ("v3", "v4"):
        s = DveOpSpec(
            name=_OP2_NAME, opcode=row, uops=_build_seg_uops(spec, ver), rd1_en=True
        )
        shas[ver] = s.sha(ver)
    op = _SegDveOp(_OP2_NAME, spec, subdim=True, uops_sha=shas)
    dve_ops.OPS.append(op)
    dve_ops.CUSTOM_DVE_SPECS[_OP2_NAME] = spec
    dve_ops._SUB_OPCODE_FOR_NAME[_OP2_NAME] = row
    return op


def _register_custom_op():
    """Register scan(ADD, Src0*Src1) as a custom DVE op (runtime-local)."""
    from concourse import dve_ops
    from concourse.dve_spec import AluOp, Spec, Src0, Src1, _has_src1, lower, scan
    from concourse.dve_uop import DveOpSpec

    for op in dve_ops.OPS:
        if op.name == _OP_NAME:
            return op

    def _ref(in0, in1, s0, s1, imm2):
        p = (np.asarray(in0, np.float32) * np.asarray(in1, np.float32)).astype(
            np.float32
        )
        shp = p.shape
        return (
            np.cumsum(p.reshape(shp[0], -1), axis=1, dtype=np.float32).reshape(shp)
        )

    spec = Spec(body=scan(AluOp.ADD, Src0 * Src1), reference=_ref)
    row = dve_ops._CUSTOM_DVE_ROW_BASE + len(dve_ops.OPS)
    shas = {}
    for ver in ("v3", "v4"):
        s = DveOpSpec(
            name=_OP_NAME, opcode=row, uops=lower(spec, ver=ver), rd1_en=_has_src1(spec)
        )
        shas[ver] = s.sha(ver)
    op = dve_ops.DveOp(_OP_NAME, spec, subdim=False, uops_sha=shas)
    dve_ops.OPS.append(op)
    dve_ops.CUSTOM_DVE_SPECS[_OP_NAME] = spec
    dve_ops._SUB_OPCODE_FOR_NAME[_OP_NAME] = row
    return op


def _build(b_shard, in_f, out_f, n_cores, x_bufs=4, halves=4, n_prologue=0, tail_quarters=4):
    """Build + compile the per-core Bass module (SPMD across n_cores)."""
    from concourse import bacc, tile, mybir

    op = _register_custom_op()
    op2 = _register_seg_op()

    k = K
    n_tiles = b_shard // P
    hw = in_f // halves  # half-tile width (multiple of K)
    hy = hw // k
    f16 = mybir.dt.float16

    nc = bacc.Bacc(
        "TRN2",
        target_bir_lowering=False,
        debug=False,
        enable_asserts=True,
        num_devices=n_cores,
    )
    x_d = nc.dram_tensor("x", [b_shard, in_f], f16, kind="ExternalInput")
    w_d = nc.dram_tensor("w", [1, in_f], f16, kind="ExternalInput")
    y_d = nc.dram_tensor("y", [b_shard, out_f], f16, kind="ExternalOutput")

    with tile.TileContext(nc) as tc:
        with (
            tc.tile_pool(name="consts", bufs=1) as cpool,
            tc.tile_pool(name="work", bufs=x_bufs) as pool,
            tc.tile_pool(name="outs", bufs=3) as ypool,
            tc.tile_pool(name="tailq", bufs=4) as qpool,
        ):
            wb = cpool.tile([P, in_f], f16, tag="w")
            # w first in the Sync HWDGE FIFO: its 32KB completes ~5us
            # earlier than via SWDGE (GpSimd's preamble delays emission),
            # and it only displaces x0's issue by ~0.7us.
            nc.sync.dma_start(out=wb[0:1, :], in_=w_d[:])
            for h in range(halves):
                nc.gpsimd.partition_broadcast(
                    wb[:, h * hw : (h + 1) * hw], wb[0:1, h * hw : (h + 1) * hw]
                )
            def chunk(i, xap, c0, cw):
                """Process columns [c0, c0+cw) of row-block i from AP xap."""
                rows = slice(i * P, (i + 1) * P)
                cg = cw // k  # groups in this chunk
                # One instruction per chunk: segmented multiply-scan with a
                # hardware page reset (SUB_DIM_DONE step state) over in0's
                # [P, cg, K] access pattern.  The out AP has innermost
                # stride 0 over each group's K elements, so the last write
                # (the completed group sum) survives, laid out contiguously.
                yt = ypool.tile([P, cg], f16, tag="s")
                y_view = yt[:].rearrange("p (g o) -> p g o", o=1).broadcast_to(
                    [P, cg, k]
                )
                nc.vector._custom_dve(
                    op2,
                    out=y_view,
                    in0=xap.rearrange("p (g kk) -> p g kk", kk=k),
                    in1=wb[:, c0 : c0 + cw],
                )
                # y stores ride the ScalarE HWDGE queue so their semaphore
                # waits never block the x-load issue stream (HWDGE is FIFO
                # per issuing engine).
                nc.scalar.dma_start(
                    out=y_d[rows, c0 // k : (c0 + cw) // k], in_=yt[:]
                )

            for i in range(n_tiles):
                rows = slice(i * P, (i + 1) * P)
                if i == n_tiles - 1 and tail_quarters > 1:
                    # split the final tile so the post-stream tail is short
                    qw = in_f // tail_quarters
                    for q in range(tail_quarters):
                        xt = qpool.tile([P, qw], f16, tag="xq")
                        nc.sync.dma_start(
                            out=xt[:], in_=x_d[rows, q * qw : (q + 1) * qw]
                        )
                        chunk(i, xt[:], q * qw, qw)
                else:
                    if i < n_prologue:
                        # dedicated startup buffers: extra DMA runway at start
                        xt = cpool.tile([P, in_f], f16, tag=f"xpro{i}")
                    else:
                        xt = pool.tile([P, in_f], f16, tag="x")
                    nc.sync.dma_start(out=xt[:], in_=x_d[rows, :])
                    if i == 0 and halves > 1:
                        # quarter-scans against matching wb ranges: each
                        # gates on its own partial broadcast, starting
                        # compute ~9us earlier (no extra bytes moved)
                        for q in range(halves):
                            chunk(i, xt[:, q * hw : (q + 1) * hw], q * hw, hw)
                    else:
                        chunk(i, xt[:], 0, in_f)
    nc.compile()
    return nc


def _prep_weights(log_weight, out_f, k):
    w = np.exp(np.asarray(log_weight, np.float64)).reshape(1, -1)  # [1, out_f*k]
    return np.ascontiguousarray(w, dtype=np.float16)


def kernel(x, log_weight):
    from concourse import bass_utils

    x = np.ascontiguousarray(np.asarray(x, dtype=np.float16))
    assert x.shape == (B, IN_F), x.shape
    b_shard = B // N_CORES

    if "nc" not in _CACHE:
        _CACHE["nc"] = _build(b_shard, IN_F, OUT_F, N_CORES)
    nc = _CACHE["nc"]

    wb = _prep_weights(log_weight, OUT_F, K)
    in_maps = [
        {"x": x[i * b_shard : (i + 1) * b_shard], "w": wb}
        for i in range(N_CORES)
    ]
    res = bass_utils.run_bass_kernel_spmd(nc, in_maps, core_ids=list(range(N_CORES)))
    y = np.concatenate([res.results[i]["y"] for i in range(N_CORES)], axis=0)
    return y.astype(np.float32)



# revision 9
# speedup vs baseline: 2.0700x; 1.7564x over previous
"""Trainium2 Bass kernel for nn_BlockLinear forward.

Computes y[b, o] = sum_k exp(log_weight[o, k]) * x[b, o*K + k]
for x [16384, 8192], log_weight [1024, 8] (fp32 interface).

Strategy: data-parallel over batch across 8 NeuronCores (2048 rows each),
fp16 on the wire (the 2e-2 rel-err gate leaves ~40x headroom over fp16's
~5e-4).  Host casts x to fp16 and replicates w=exp(log_weight) to
[128, 8192] fp16; the device streams 16 tiles of [128, 8192] per core and
runs ONE custom DVE instruction per tile:

    y[p, g] = sum_k x[p, 8g+k] * w[p, 8g+k]

The custom op (SEGSUM8_PAIR2X_ANT) is a COUNT-driven uop FSM (K=8 is
static) with both a 1x program and a hand-packed 2x_1PORT program.  In 2x
mode the DVE consumes two packed fp16 elements per cycle (SRC/SRC_HI
lanes): blocks 0/1 form the two products, block 2 adds the pair, block 3
holds the running group sum in its CURR_ALU_OUT flop (II=1).  Group sums
are emitted in PAIRS: the even group's sum is parked in block 3's swap
flop during the odd group's reset uop, and the odd group's last pair
writes {WR0_LO=even, WR0_HI=odd} - one aligned 4-byte fp16x2 store per 8
cycles, satisfying the 2x dst preconditions (dense, step 1, 2B dtype).

Engine budget per tile: DVE 4.3us (2x) vs DMA 6.6us (2 MiB x + 0.25 MiB
y at ~358 GB/s HBM-per-NC) -> memory-bound at the fp16 roofline.  Loads
ride the Sync HWDGE queue, stores the ScalarE HWDGE queue (FIFO per
engine, so store sem-waits never block load issue).  w quarters interleave
with x-tile-0 quarters at the head of the Sync FIFO so the first scans
start ~4us in; the last tile is quarter-split to shorten the drain.
"""

import numpy as np

B = 16384
IN_F = 8192
OUT_F = 1024
K = 8
N_CORES = 8
P = 128

_CACHE = {}

_OP_NAME = "SEGSUM8_PAIR2X_ANT"


def _build_pair_uops():
    """Build (uops_1x, uops_2x) for the grouped (K=8) multiply-reduce.

    Both programs are COUNT-driven (no SUB_DIM triggers): the group
    structure is static, so the FSM loops on element counts.  next_uop
    index 0 means IDLE, so the steady-state loop lives at indices >= 1
    and the entry uop at index 0 is a one-shot copy of the loop head.
    """
    from concourse.dve_uop import (
        ENABLE,
        AluInp,
        AluOp,
        DelayInp,
        InpSel,
        OutPath,
        OutSel,
        Trigger,
        UopConfig,
        UopDpConfig,
    )

    def chain(u, rep, nxt):
        u.repeat_count = rep
        u.trigger = (Trigger.SRC_TENSOR_DONE, Trigger.COUNT, Trigger.NONE)
        u.next_uop = (0, nxt, 0)
        u.require_inp0 = ENABLE
        u.require_inp1 = ENABLE
        return u

    # ---- 1x program: 1 elem/cycle, acc in block1, write once per group ----
    def u1x(kind, write):
        u = UopConfig()
        u.enable_input(InpSel.SRC_0, 0)  # x   -> block0 ALU A
        u.enable_input(InpSel.SRC_1, 1)  # w   -> block0 PREV_DELAY_0
        u.datapath_config[0] = UopDpConfig().enable_alu(
            AluOp.MULTIPLY, AluInp.PREV_ALU_OUT, AluInp.PREV_DELAY_0
        )
        op = AluOp.BYPASS if kind == "reset" else AluOp.ADD
        u.datapath_config[1] = UopDpConfig().enable_alu(
            op, AluInp.PREV_ALU_OUT, AluInp.CURR_ALU_OUT
        )
        for b in range(2, 8):
            u.datapath_config[b] = UopDpConfig().pass_through_alu()
        if write:
            u.enable_output(OutSel.ALU_OUT, OutPath.WR0_LO)
        return u

    # reset(1) -> acc(6) -> emit(1,write) -> reset(1) -> acc(6) -> emit -> ...
    ops_1x = [
        chain(u1x("reset", False), 1, 1),  # 0: entry reset
        chain(u1x("acc", False), 6, 2),    # 1
        chain(u1x("acc", True), 1, 3),     # 2: emit (8th element completes sum)
        chain(u1x("reset", False), 1, 4),  # 3: loop reset
        chain(u1x("acc", False), 6, 5),    # 4
        chain(u1x("acc", True), 1, 3),     # 5: emit -> loop reset
    ]

    # ---- 2x_1PORT program: 2 packed fp16/cycle, acc in block3 --------------
    def u2x(kind):
        u = UopConfig()
        u.enable_input(InpSel.SRC_0, 0)     # x_lo -> block0 ALU A
        u.enable_input(InpSel.SRC_1, 1)     # w_lo -> PREV_DELAY_0
        u.enable_input(InpSel.SRC_0_HI, 2)  # x_hi -> PREV_DELAY_1
        u.enable_input(InpSel.SRC_1_HI, 3)  # w_hi -> PREV_DELAY_2
        u.datapath_config[0] = (
            UopDpConfig()
            .enable_alu(AluOp.MULTIPLY, AluInp.PREV_ALU_OUT, AluInp.PREV_DELAY_0)
            .pass_through_delay(1, 2)
        )
        u.datapath_config[1] = (
            UopDpConfig()
            .enable_alu(AluOp.MULTIPLY, AluInp.PREV_DELAY_1, AluInp.PREV_DELAY_2)
            .enable_delay_from_src(DelayInp.PREV_ALU_OUT, 0)  # m_lo
        )
        u.datapath_config[2] = UopDpConfig().enable_alu(
            AluOp.ADD, AluInp.PREV_ALU_OUT, AluInp.PREV_DELAY_0  # t = m_hi+m_lo
        )
        b3 = UopDpConfig()
        if kind in ("reset", "reset_hold"):
            # drop the CURR feedback: acc <- t (new group's first pair)
            b3.enable_alu(AluOp.BYPASS, AluInp.PREV_ALU_OUT, AluInp.CURR_ALU_OUT)
            if kind == "reset_hold":
                # capture the completed even-group sum (block3's CURR flop)
                # into delay lane 4; later uops leave d4 disabled at block3,
                # so the flop HOLDS it until the emit element reads it.
                b3.enable_delay_from_src(DelayInp.CURR_ALU_OUT, 4)
        else:
            b3.enable_alu(AluOp.ADD, AluInp.PREV_ALU_OUT, AluInp.CURR_ALU_OUT)
        u.datapath_config[3] = b3
        for b in range(4, 8):
            cfg = UopDpConfig().pass_through_alu()
            if kind in ("emit", "acc", "reset_hold"):
                # keep the held even sum flowing toward the output flops
                cfg.pass_through_delay(4)
            u.datapath_config[b] = cfg
        if kind == "emit":
            # one packed 4B write per pair of groups: {lo=even sum, hi=odd sum}
            u.enable_output(OutSel.DELAY_4, OutPath.WR0_LO)
            u.enable_output(OutSel.ALU_OUT, OutPath.WR0_HI)
        return u

    # repeat_count counts ISSUE CYCLES (2 elements each in 2x mode):
    # resetE(1) -> accA(3) -> resetO(1, hold even sum) -> accB(2)
    #   -> emit(1, write pair) -> resetE(1) -> accA ...
    ops_2x = [
        chain(u2x("reset"), 1, 1),       # 0: entry (even group pair 0)
        chain(u2x("acc"), 3, 2),         # 1: even pairs 1-3
        chain(u2x("reset_hold"), 1, 3),  # 2: odd pair 0, park even sum in d4
        chain(u2x("acc"), 2, 4),         # 3: odd pairs 1-2
        chain(u2x("emit"), 1, 5),        # 4: odd pair 3, write {even,odd}
        chain(u2x("reset"), 1, 1),       # 5: loop reset (next even group)
    ]
    return ops_1x, ops_2x


def _register_pair_op():
    """Register SEGSUM8_PAIR2X_ANT (1x + 2x_1PORT programs, perf_max=1)."""
    import dataclasses

    from concourse import dve_ops
    from concourse.dve_spec import AluOp, Spec, Src0, Src1, scan
    from concourse.dve_uop import DveOpSpec

    for op in dve_ops.OPS:
        if op.name == _OP_NAME:
            return op

    def _ref(in0, in1, s0, s1, imm2):
        p = np.asarray(in0, np.float32) * np.asarray(in1, np.float32).reshape(
            np.asarray(in0).shape
        )
        return p.reshape(p.shape[0], -1, K).sum(axis=-1, dtype=np.float32)

    # body is structural only (Src1 presence -> rd1_en); semantics live in
    # the hand-built uop programs + `_ref` (used by the interpreter).
    spec = Spec(body=scan(AluOp.ADD, Src0 * Src1), reference=_ref)

    @dataclasses.dataclass(frozen=True)
    class _PairDveOp(dve_ops.DveOp):
        def compile(self, ver):
            key = (self.name, ver)
            cached = dve_ops._COMPILE_CACHE.get(key)
            if cached is not None:
                return cached
            u1x, u2x = _build_pair_uops()
            result = DveOpSpec(
                name=self.name,
                opcode=dve_ops.get_dve_sub_opcode(self.name),
                uops=u1x,
                uops_2x=u2x,
                perf_max=1,
                rd1_en=True,
            )
            result.validate(ver)
            dve_ops._COMPILE_CACHE[key] = result
            return result

    row = dve_ops._CUSTOM_DVE_ROW_BASE + len(dve_ops.OPS)
    op = _PairDveOp(_OP_NAME, spec, subdim=False, uops_sha={})
    dve_ops.OPS.append(op)
    dve_ops.CUSTOM_DVE_SPECS[_OP_NAME] = spec
    dve_ops._SUB_OPCODE_FOR_NAME[_OP_NAME] = row
    return op


def _build(b_shard, in_f, out_f, n_cores, x_bufs=4, quarters=4, tail_quarters=4):
    """Build + compile the per-core Bass module (SPMD across n_cores)."""
    from concourse import bacc, mybir, tile

    op = _register_pair_op()

    k = K
    n_tiles = b_shard // P
    qw = in_f // quarters  # quarter width (multiple of 16)
    f16 = mybir.dt.float16

    nc = bacc.Bacc(
        "TRN2",
        target_bir_lowering=False,
        debug=False,
        enable_asserts=True,
        num_devices=n_cores,
    )
    x_d = nc.dram_tensor("x", [b_shard, in_f], f16, kind="ExternalInput")
    w_d = nc.dram_tensor("w", [P, in_f], f16, kind="ExternalInput")
    y_d = nc.dram_tensor("y", [b_shard, out_f], f16, kind="ExternalOutput")

    with tile.TileContext(nc) as tc:
        with (
            tc.tile_pool(name="consts", bufs=1) as cpool,
            tc.tile_pool(name="work", bufs=x_bufs) as pool,
            tc.tile_pool(name="outs", bufs=3) as ypool,
            tc.tile_pool(name="tailq", bufs=4) as qpool,
        ):
            wb = cpool.tile([P, in_f], f16, tag="w")

            def chunk(i, xap, c0, cw):
                """Grouped multiply-reduce on columns [c0, c0+cw) of tile i."""
                rows = slice(i * P, (i + 1) * P)
                cg = cw // k
                yt = ypool.tile([P, cg], f16, tag="s")
                ins = nc.vector._custom_dve(
                    op,
                    out=yt[:],
                    in0=xap,
                    in1=wb[:, c0 : c0 + cw],
                )
                ins.ins.perf_max = 1  # byte-36[7:6]: allow the 2x_1PORT slot
                # y stores ride the ScalarE HWDGE queue so their semaphore
                # waits never block the x-load issue stream.
                nc.scalar.dma_start(
                    out=y_d[rows, c0 // k : (c0 + cw) // k], in_=yt[:]
                )

            for i in range(n_tiles):
                rows = slice(i * P, (i + 1) * P)
                if i == 0:
                    # interleave w quarters with x0 quarters on the Sync
                    # FIFO: quarter-scan q only needs {w_q, x0_q}, so the
                    # first scan starts ~4us in instead of ~8us.
                    xt = pool.tile([P, in_f], f16, tag="x")
                    for q in range(quarters):
                        cs = slice(q * qw, (q + 1) * qw)
                        nc.sync.dma_start(out=wb[:, cs], in_=w_d[:, cs])
                        nc.sync.dma_start(out=xt[:, cs], in_=x_d[rows, cs])
                        chunk(i, xt[:, cs], q * qw, qw)
                elif i == n_tiles - 1 and tail_quarters > 1:
                    # split the final tile so the post-stream tail is short
                    tqw = in_f // tail_quarters
                    for q in range(tail_quarters):
                        xq = qpool.tile([P, tqw], f16, tag="xq")
                        nc.sync.dma_start(
                            out=xq[:], in_=x_d[rows, q * tqw : (q + 1) * tqw]
                        )
                        chunk(i, xq[:], q * tqw, tqw)
                else:
                    xt = pool.tile([P, in_f], f16, tag="x")
                    nc.sync.dma_start(out=xt[:], in_=x_d[rows, :])
                    chunk(i, xt[:], 0, in_f)
    nc.compile()
    return nc


def _prep_weights(log_weight):
    w = np.exp(np.asarray(log_weight, np.float64)).reshape(1, -1)  # [1, out_f*k]
    return np.ascontiguousarray(np.broadcast_to(w.astype(np.float16), (P, w.size)))


def kernel(x, log_weight):
    from concourse import bass_utils

    x = np.ascontiguousarray(np.asarray(x).astype(np.float16))
    assert x.shape == (B, IN_F), x.shape
    b_shard = B // N_CORES

    if "nc" not in _CACHE:
        _CACHE["nc"] = _build(b_shard, IN_F, OUT_F, N_CORES)
    nc = _CACHE["nc"]

    wb = _prep_weights(log_weight)
    in_maps = [
        {"x": x[i * b_shard : (i + 1) * b_shard], "w": wb}
        for i in range(N_CORES)
    ]
    res = bass_utils.run_bass_kernel_spmd(nc, in_maps, core_ids=list(range(N_CORES)))
    y = np.concatenate([res.results[i]["y"] for i in range(N_CORES)], axis=0)
    return y.astype(np.float32)


# revision 12
# speedup vs baseline: 2.1195x; 1.0239x over previous
"""Trainium2 Bass kernel for nn_BlockLinear forward.

Computes y[b, o] = sum_k exp(log_weight[o, k]) * x[b, o*K + k]
for x [16384, 8192], log_weight [1024, 8] (fp32 interface).

Strategy: data-parallel over batch across 8 NeuronCores (2048 rows each),
fp16 on the wire (the 2e-2 rel-err gate leaves ~40x headroom over fp16's
~5e-4).  Host casts x to fp16 and replicates w=exp(log_weight) to
[128, 8192] fp16; the device streams 16 tiles of [128, 8192] per core and
runs ONE custom DVE instruction per tile:

    y[p, g] = sum_k x[p, 8g+k] * w[p, 8g+k]

The custom op (SEGSUM8_PAIR2X_ANT) is a COUNT-driven uop FSM (K=8 is
static) with both a 1x program and a hand-packed 2x_1PORT program.  In 2x
mode the DVE consumes two packed fp16 elements per cycle (SRC/SRC_HI
lanes): blocks 0/1 form the two products, block 2 adds the pair, block 3
holds the running group sum in its CURR_ALU_OUT flop (II=1).  Group sums
are emitted in PAIRS: the even group's sum is parked in block 3's swap
flop during the odd group's reset uop, and the odd group's last pair
writes {WR0_LO=even, WR0_HI=odd} - one aligned 4-byte fp16x2 store per 8
cycles, satisfying the 2x dst preconditions (dense, step 1, 2B dtype).

Engine budget per tile: DVE 4.3us (2x) vs DMA 6.6us (2 MiB x + 0.25 MiB
y at ~358 GB/s HBM-per-NC) -> memory-bound at the fp16 roofline.  Loads
ride the Sync HWDGE queue, stores the ScalarE HWDGE queue (FIFO per
engine, so store sem-waits never block load issue).  w quarters interleave
with x-tile-0 quarters at the head of the Sync FIFO so the first scans
start ~4us in; the last tile is quarter-split to shorten the drain.
"""

import numpy as np

B = 16384
IN_F = 8192
OUT_F = 1024
K = 8
N_CORES = 8
P = 128

_CACHE = {}

_OP_NAME = "SEGSUM8_PAIR2X_ANT"


def _build_pair_uops():
    """Build (uops_1x, uops_2x) for the grouped (K=8) multiply-reduce.

    Both programs are COUNT-driven (no SUB_DIM triggers): the group
    structure is static, so the FSM loops on element counts.  next_uop
    index 0 means IDLE, so the steady-state loop lives at indices >= 1
    and the entry uop at index 0 is a one-shot copy of the loop head.
    """
    from concourse.dve_uop import (
        ENABLE,
        AluInp,
        AluOp,
        DelayInp,
        InpSel,
        OutPath,
        OutSel,
        Trigger,
        UopConfig,
        UopDpConfig,
    )

    def chain(u, rep, nxt):
        u.repeat_count = rep
        u.trigger = (Trigger.SRC_TENSOR_DONE, Trigger.COUNT, Trigger.NONE)
        u.next_uop = (0, nxt, 0)
        u.require_inp0 = ENABLE
        u.require_inp1 = ENABLE
        return u

    # ---- 1x program: 1 elem/cycle, acc in block1, write once per group ----
    def u1x(kind, write):
        u = UopConfig()
        u.enable_input(InpSel.SRC_0, 0)  # x   -> block0 ALU A
        u.enable_input(InpSel.SRC_1, 1)  # w   -> block0 PREV_DELAY_0
        u.datapath_config[0] = UopDpConfig().enable_alu(
            AluOp.MULTIPLY, AluInp.PREV_ALU_OUT, AluInp.PREV_DELAY_0
        )
        op = AluOp.BYPASS if kind == "reset" else AluOp.ADD
        u.datapath_config[1] = UopDpConfig().enable_alu(
            op, AluInp.PREV_ALU_OUT, AluInp.CURR_ALU_OUT
        )
        for b in range(2, 8):
            u.datapath_config[b] = UopDpConfig().pass_through_alu()
        if write:
            u.enable_output(OutSel.ALU_OUT, OutPath.WR0_LO)
        return u

    # reset(1) -> acc(6) -> emit(1,write) -> reset(1) -> acc(6) -> emit -> ...
    ops_1x = [
        chain(u1x("reset", False), 1, 1),  # 0: entry reset
        chain(u1x("acc", False), 6, 2),    # 1
        chain(u1x("acc", True), 1, 3),     # 2: emit (8th element completes sum)
        chain(u1x("reset", False), 1, 4),  # 3: loop reset
        chain(u1x("acc", False), 6, 5),    # 4
        chain(u1x("acc", True), 1, 3),     # 5: emit -> loop reset
    ]

    # ---- 2x_1PORT program: 2 packed fp16/cycle, acc in block3 --------------
    def u2x(kind):
        u = UopConfig()
        u.enable_input(InpSel.SRC_0, 0)     # x_lo -> block0 ALU A
        u.enable_input(InpSel.SRC_1, 1)     # w_lo -> PREV_DELAY_0
        u.enable_input(InpSel.SRC_0_HI, 2)  # x_hi -> PREV_DELAY_1
        u.enable_input(InpSel.SRC_1_HI, 3)  # w_hi -> PREV_DELAY_2
        u.datapath_config[0] = (
            UopDpConfig()
            .enable_alu(AluOp.MULTIPLY, AluInp.PREV_ALU_OUT, AluInp.PREV_DELAY_0)
            .pass_through_delay(1, 2)
        )
        u.datapath_config[1] = (
            UopDpConfig()
            .enable_alu(AluOp.MULTIPLY, AluInp.PREV_DELAY_1, AluInp.PREV_DELAY_2)
            .enable_delay_from_src(DelayInp.PREV_ALU_OUT, 0)  # m_lo
        )
        u.datapath_config[2] = UopDpConfig().enable_alu(
            AluOp.ADD, AluInp.PREV_ALU_OUT, AluInp.PREV_DELAY_0  # t = m_hi+m_lo
        )
        b3 = UopDpConfig()
        if kind in ("reset", "reset_hold"):
            # drop the CURR feedback: acc <- t (new group's first pair)
            b3.enable_alu(AluOp.BYPASS, AluInp.PREV_ALU_OUT, AluInp.CURR_ALU_OUT)
            if kind == "reset_hold":
                # capture the completed even-group sum (block3's CURR flop)
                # into delay lane 4; later uops leave d4 disabled at block3,
                # so the flop HOLDS it until the emit element reads it.
                b3.enable_delay_from_src(DelayInp.CURR_ALU_OUT, 4)
        else:
            b3.enable_alu(AluOp.ADD, AluInp.PREV_ALU_OUT, AluInp.CURR_ALU_OUT)
        u.datapath_config[3] = b3
        for b in range(4, 8):
            cfg = UopDpConfig().pass_through_alu()
            if kind in ("emit", "acc", "reset_hold"):
                # keep the held even sum flowing toward the output flops
                cfg.pass_through_delay(4)
            u.datapath_config[b] = cfg
        if kind == "emit":
            # one packed 4B write per pair of groups: {lo=even sum, hi=odd sum}
            u.enable_output(OutSel.DELAY_4, OutPath.WR0_LO)
            u.enable_output(OutSel.ALU_OUT, OutPath.WR0_HI)
        return u

    # repeat_count counts ISSUE CYCLES (2 elements each in 2x mode):
    # resetE(1) -> accA(3) -> resetO(1, hold even sum) -> accB(2)
    #   -> emit(1, write pair) -> resetE(1) -> accA ...
    ops_2x = [
        chain(u2x("reset"), 1, 1),       # 0: entry (even group pair 0)
        chain(u2x("acc"), 3, 2),         # 1: even pairs 1-3
        chain(u2x("reset_hold"), 1, 3),  # 2: odd pair 0, park even sum in d4
        chain(u2x("acc"), 2, 4),         # 3: odd pairs 1-2
        chain(u2x("emit"), 1, 5),        # 4: odd pair 3, write {even,odd}
        chain(u2x("reset"), 1, 1),       # 5: loop reset (next even group)
    ]
    return ops_1x, ops_2x


def _register_pair_op():
    """Register SEGSUM8_PAIR2X_ANT (1x + 2x_1PORT programs, perf_max=1)."""
    import dataclasses

    from concourse import dve_ops
    from concourse.dve_spec import AluOp, Spec, Src0, Src1, scan
    from concourse.dve_uop import DveOpSpec

    for op in dve_ops.OPS:
        if op.name == _OP_NAME:
            return op

    def _ref(in0, in1, s0, s1, imm2):
        p = np.asarray(in0, np.float32) * np.asarray(in1, np.float32).reshape(
            np.asarray(in0).shape
        )
        return p.reshape(p.shape[0], -1, K).sum(axis=-1, dtype=np.float32)

    # body is structural only (Src1 presence -> rd1_en); semantics live in
    # the hand-built uop programs + `_ref` (used by the interpreter).
    spec = Spec(body=scan(AluOp.ADD, Src0 * Src1), reference=_ref)

    @dataclasses.dataclass(frozen=True)
    class _PairDveOp(dve_ops.DveOp):
        def compile(self, ver):
            key = (self.name, ver)
            cached = dve_ops._COMPILE_CACHE.get(key)
            if cached is not None:
                return cached
            u1x, u2x = _build_pair_uops()
            result = DveOpSpec(
                name=self.name,
                opcode=dve_ops.get_dve_sub_opcode(self.name),
                uops=u1x,
                uops_2x=u2x,
                perf_max=1,
                rd1_en=True,
            )
            result.validate(ver)
            dve_ops._COMPILE_CACHE[key] = result
            return result

    row = dve_ops._CUSTOM_DVE_ROW_BASE + len(dve_ops.OPS)
    op = _PairDveOp(_OP_NAME, spec, subdim=False, uops_sha={})
    dve_ops.OPS.append(op)
    dve_ops.CUSTOM_DVE_SPECS[_OP_NAME] = spec
    dve_ops._SUB_OPCODE_FOR_NAME[_OP_NAME] = row
    return op


def _build(b_shard, in_f, out_f, n_cores, x_bufs=4, quarters=4, tail_quarters=4):
    """Build + compile the per-core Bass module (SPMD across n_cores)."""
    from concourse import bacc, mybir, tile

    op = _register_pair_op()

    k = K
    n_tiles = b_shard // P
    qw = in_f // quarters  # quarter width (multiple of 16)
    f16 = mybir.dt.float16

    nc = bacc.Bacc(
        "TRN2",
        target_bir_lowering=False,
        debug=False,
        enable_asserts=True,
        num_devices=n_cores,
    )
    x_d = nc.dram_tensor("x", [b_shard, in_f], f16, kind="ExternalInput")
    w_d = nc.dram_tensor("w", [1, in_f], f16, kind="ExternalInput")
    y_d = nc.dram_tensor("y", [b_shard, out_f], f16, kind="ExternalOutput")

    with tile.TileContext(nc) as tc:
        with (
            tc.tile_pool(name="consts", bufs=1) as cpool,
            tc.tile_pool(name="work", bufs=x_bufs) as pool,
            tc.tile_pool(name="outs", bufs=3) as ypool,
            tc.tile_pool(name="tailq", bufs=4) as qpool,
            tc.psum_pool(name="wpsum", bufs=2) as ppool,
        ):
            wb = cpool.tile([P, in_f], f16, tag="w")
            w_sb = cpool.tile([1, in_f], f16, tag="w_row")
            ones = cpool.tile([1, P], f16, tag="ones")

            def chunk(i, xap, c0, cw):
                """Grouped multiply-reduce on columns [c0, c0+cw) of tile i."""
                rows = slice(i * P, (i + 1) * P)
                cg = cw // k
                yt = ypool.tile([P, cg], f16, tag="s")
                ins = nc.vector._custom_dve(
                    op,
                    out=yt[:],
                    in0=xap,
                    in1=wb[:, c0 : c0 + cw],
                )
                ins.ins.perf_max = 1  # byte-36[7:6]: allow the 2x_1PORT slot
                # y stores ride the ScalarE HWDGE queue so their semaphore
                # waits never block the x-load issue stream.
                nc.scalar.dma_start(
                    out=y_d[rows, c0 // k : (c0 + cw) // k], in_=yt[:]
                )

            # w broadcast [1, in_f] -> [128, in_f] OFF the HBM stream: a
            # 16 KiB w load rides first on the Sync FIFO, then the (idle)
            # PE replicates it into PSUM chunks via a ones[1,128] matmul
            # (K=1 contraction) and the (idle) ACT engine copies each
            # PSUM chunk to wb as fp16.  Saves 2 MiB of HBM traffic and
            # ~6us of stream time vs a host-replicated w load.
            nc.sync.dma_start(out=w_sb[:], in_=w_d[:])
            nc.vector.memset(ones[:], 1.0)
            mm = 512  # PE moving-free-dim max; [P, 512] fp32 = 1 PSUM bank
            for c in range(in_f // mm):
                cs = slice(c * mm, (c + 1) * mm)
                pt = ppool.tile([P, mm], mybir.dt.float32, tag="wp")
                nc.tensor.matmul(
                    pt[:], ones[0:1, :], w_sb[0:1, cs], start=True, stop=True
                )
                nc.scalar.copy(out=wb[:, cs], in_=pt[:])

            for i in range(n_tiles):
                rows = slice(i * P, (i + 1) * P)
                if i == 0:
                    # x0 in quarters: quarter-scan q gates only on its own
                    # wb range + x0 quarter, so scans start early.
                    xt = pool.tile([P, in_f], f16, tag="x")
                    for q in range(quarters):
                        cs = slice(q * qw, (q + 1) * qw)
                        nc.sync.dma_start(out=xt[:, cs], in_=x_d[rows, cs])
                        chunk(i, xt[:, cs], q * qw, qw)
                elif i == n_tiles - 1 and tail_quarters > 1:
                    # split the final tile so the post-stream tail is short
                    tqw = in_f // tail_quarters
                    for q in range(tail_quarters):
                        xq = qpool.tile([P, tqw], f16, tag="xq")
                        nc.sync.dma_start(
                            out=xq[:], in_=x_d[rows, q * tqw : (q + 1) * tqw]
                        )
                        chunk(i, xq[:], q * tqw, tqw)
                else:
                    xt = pool.tile([P, in_f], f16, tag="x")
                    nc.sync.dma_start(out=xt[:], in_=x_d[rows, :])
                    chunk(i, xt[:], 0, in_f)
    nc.compile()
    return nc


def _prep_weights(log_weight):
    w = np.exp(np.asarray(log_weight, np.float64)).reshape(1, -1)  # [1, out_f*k]
    return np.ascontiguousarray(w.astype(np.float16))


def kernel(x, log_weight):
    from concourse import bass_utils

    x = np.ascontiguousarray(np.asarray(x).astype(np.float16))
    assert x.shape == (B, IN_F), x.shape
    b_shard = B // N_CORES

    if "nc" not in _CACHE:
        _CACHE["nc"] = _build(b_shard, IN_F, OUT_F, N_CORES)
    nc = _CACHE["nc"]

    wb = _prep_weights(log_weight)
    in_maps = [
        {"x": x[i * b_shard : (i + 1) * b_shard], "w": wb}
        for i in range(N_CORES)
    ]
    res = bass_utils.run_bass_kernel_spmd(nc, in_maps, core_ids=list(range(N_CORES)))
    y = np.concatenate([res.results[i]["y"] for i in range(N_CORES)], axis=0)
    return y.astype(np.float32)
